# revision 2
# baseline (speedup 1.0000x reference)
"""GCN layer (GCNConv + PReLU) on TRN2, SPMD across 8 NeuronCores — v2.

out = PReLU(A_hat @ (x W) + b), A_hat = D^-1/2 (A+I) D^-1/2. Using
commutativity: out = PReLU((A_hat @ xs) W + b) with xs = dinv*x gathered
per edge (bf16), dinv_dst applied in the one-hot scatter, and the +I
self-loop term computed from a locally streamed block (identity matmul,
no gather).

Per core (12500 dst nodes, 98 windows of 128):
  - edges sorted by dst window (dealt to slots for SPMD balance), then by
    src chunk (4 chunks of 25000 rows so indices fit dma_gather's int16).
  - gpsimd: dma_gather calls (<=8 tiles each, 4 SWDGE queues round-robin)
    pull xs rows (bf16) into 16 rotating staging buffers. ~4x faster than
    per-tile indirect_dma_start: the gather path is descriptor-rate-bound
    and the 4 queues generate descriptors in parallel.
  - DVE: H[e, j] = (iota==dstloc_e) * dinv_dst_e per tile (bf16).
  - PE: accT[feat, dstloc] += rows^T @ H per tile (bf16, 4x fp32 rate);
    self-loops via accT += xblk_w^T @ I with xblk = dinv^2*x (bf16);
    z^T[hid, dstloc] = W^T @ accT (fp32).
  - ACT: PSUM accT -> SBUF copy; fused bias+PReLU via activation(Prelu,
    bias=b, alpha=a) straight out of PSUM.
  - sync (HWDGE): bulk loads at start, per-window 64KB stores of z^T.

All scheduling is manual: one counting semaphore per engine plus 16 DMA
sems; wait thresholds are compile-time constants precomputed from the
static call/tile layout by a python dry run before emission.
"""

from contextlib import ExitStack

import numpy as np

import concourse.bacc as bacc
import concourse.bass as bass
import concourse.mybir as mybir
from concourse.library_config import mlp

P = 128
N_CORES = 8
N_NODES = 100000
N_CHUNKS = 4
CHUNK_ROWS = 25000  # <= 32767 so src indices fit int16
MAX_CALL_TILES = 2  # small calls pipeline ~5x better through the SWDGE rings
N_STAGE = 48  # staging buffers (2 tiles each)
N_HBUF = 64
N_QUEUES = 4

F32 = mybir.dt.float32
BF16 = mybir.dt.bfloat16
I16 = mybir.dt.int16
AF = mybir.ActivationFunctionType


def _layout(win_tiles):
    """Static schedule layout + precomputed semaphore thresholds."""
    n_win = len(win_tiles)
    n_wt = [int(sum(tc)) for tc in win_tiles]
    T = int(sum(n_wt))

    calls = []  # (chunk, start_tile, n_tiles, window)
    t0 = 0
    for w in range(n_win):
        for c in range(N_CHUNKS):
            n = int(win_tiles[w][c])
            s = t0
            while n > 0:
                k = min(n, MAX_CALL_TILES)
                calls.append((c, s, k, w))
                s += k
                n -= k
            t0 += int(win_tiles[w][c])
    n_calls = len(calls)

    win_start = np.zeros(n_win, dtype=np.int64)
    np.cumsum(n_wt[:-1], out=win_start[1:])

    buf_use = [0] * N_STAGE
    call_buf, call_use = [0] * n_calls, [0] * n_calls
    for k in range(n_calls):
        b = k % N_STAGE
        call_buf[k] = b
        buf_use[b] += 1
        call_use[k] = buf_use[b]

    tile_call = [0] * T
    for k, (c, s, nt, w) in enumerate(calls):
        for j in range(nt):
            tile_call[s + j] = k

    # ---- dry-run PE / ACT streams to fix semaphore positions ----
    pe_tile_pos = [0] * T  # pe_sem value after the matmul consuming tile t
    pe_win_last = [0] * n_win  # pe_sem value after window w's last acc matmul
    pe_wmm_pos = [0] * n_win  # pe_sem value after window w's W-matmul
    pe_call_last = [0] * n_calls  # pe_sem value freeing call k's buffer
    pos = 0
    for w in range(n_win):
        pos += 1  # diag matmul
        for j in range(n_wt[w]):
            t = int(win_start[w]) + j
            pos += 1
            pe_tile_pos[t] = pos
        pe_win_last[w] = pos
        if w >= 1:
            pos += 1
            pe_wmm_pos[w - 1] = pos
    pos += 1
    pe_wmm_pos[n_win - 1] = pos
    for k, (c, s, nt, w) in enumerate(calls):
        pe_call_last[k] = pe_tile_pos[s + nt - 1]

    # ACT stream: for w: copy_w ; if w>=1: r1_{w-1}, r2_{w-1} ; tail r1/r2
    act_copy_pos = [0] * n_win
    act_r2_pos = [0] * n_win
    apos = 0
    for w in range(n_win):
        apos += 1
        act_copy_pos[w] = apos
        if w >= 1:
            apos += 2
            act_r2_pos[w - 1] = apos
    apos += 2
    act_r2_pos[n_win - 1] = apos

    # DVE stream: for w: h tiles of w ; if w>=3: mul/sub_{w-3} ; tail.
    # The lag of 3 windows breaks the cycle PE tiles(w)->h(w+1)->DVE
    # mul/sub(v)->ACT r2(v)->PE Wmm(v) (emitted after tiles(v+1)): with
    # v=w-2 the Wmm is already behind PE's cursor.
    dve_h_pos = [0] * T
    dve_sub_pos = [0] * n_win
    dpos = 0
    for w in range(n_win):
        for j in range(n_wt[w]):
            t = int(win_start[w]) + j
            dpos += 1
            dve_h_pos[t] = dpos
        if w >= 3:
            dpos += 1
            dve_sub_pos[w - 3] = dpos
    for w in range(max(0, n_win - 3), n_win):
        dpos += 1
        dve_sub_pos[w] = dpos

    return dict(
        n_win=n_win,
        n_wt=n_wt,
        T=T,
        calls=calls,
        win_start=win_start,
        buf_use=buf_use,
        call_buf=call_buf,
        call_use=call_use,
        tile_call=tile_call,
        pe_tile_pos=pe_tile_pos,
        pe_win_last=pe_win_last,
        pe_wmm_pos=pe_wmm_pos,
        pe_call_last=pe_call_last,
        act_copy_pos=act_copy_pos,
        act_r2_pos=act_r2_pos,
        dve_h_pos=dve_h_pos,
        dve_sub_pos=dve_sub_pos,
    )


def _build_program(
    win_tiles, table_rows=N_NODES, chunk_rows=CHUNK_ROWS, mode="full", repeat=1
):
    L = _layout(win_tiles)
    n_win, n_wt, T = L["n_win"], L["n_wt"], L["T"]
    calls, win_start = L["calls"], L["win_start"]

    nc = bacc.Bacc(
        "TRN2",
        target_bir_lowering=False,
        num_swdge_queues=N_QUEUES,
    )
    x = nc.dram_tensor("x", [table_rows, P], BF16, kind="ExternalInput")
    idxs_d = nc.dram_tensor("idxs", [P, T * 8], I16, kind="ExternalInput")
    meta_d = nc.dram_tensor("meta", [P, 2 * T], F32, kind="ExternalInput")
    xblk_d = nc.dram_tensor("xblk", [P, n_win * P], BF16, kind="ExternalInput")
    cstb_d = nc.dram_tensor("cstb", [P, 2 * P], BF16, kind="ExternalInput")
    cstf_d = nc.dram_tensor("cstf", [P, P + 4], F32, kind="ExternalInput")
    y = nc.dram_tensor("y", [n_win * P, P], F32, kind="ExternalOutput")

    with ExitStack() as stack:
        ec = stack.enter_context
        block = ec(nc.Block())
        idxs_sb = ec(nc.sbuf_tensor("idxs_sb", [P, T * 8], I16))
        meta_sb = ec(nc.sbuf_tensor("meta_sb", [P, 2 * T], F32))
        xblk_sb = ec(nc.sbuf_tensor("xblk_sb", [P, n_win * P], BF16))
        cstb_sb = ec(nc.sbuf_tensor("cstb_sb", [P, 2 * P], BF16))
        cstf_sb = ec(nc.sbuf_tensor("cstf_sb", [P, P + 4], F32))
        stage = ec(nc.sbuf_tensor("stage", [P, N_STAGE * MAX_CALL_TILES, P], BF16))
        hbuf = ec(nc.sbuf_tensor("hbuf", [P, N_HBUF, P], BF16))
        asb = ec(nc.sbuf_tensor("asb", [P, 4, P], F32))
        tmp1 = ec(nc.sbuf_tensor("tmp1", [P, 4, P], F32))
        tmp2 = ec(nc.sbuf_tensor("tmp2", [P, 4, P], F32))
        outb = ec(nc.sbuf_tensor("outb", [P, 4, P], F32))
        accp = [ec(nc.psum_tensor(f"acc{i}", [P, P], F32)) for i in range(4)]
        z0 = ec(nc.psum_tensor("z0", [P, P], F32))
        z1 = ec(nc.psum_tensor("z1", [P, P], F32))
        io = ec(nc.semaphore("io"))
        pe_sem = ec(nc.semaphore("pe_sem"))
        dve_sem = ec(nc.semaphore("dve_sem"))
        act_sem = ec(nc.semaphore("act_sem"))
        ssem = [ec(nc.semaphore(f"s{i}")) for i in range(4)]
        gsem = [ec(nc.semaphore(f"g{i}")) for i in range(N_STAGE)]
        accb = accp
        zb = [z0, z1]

        iota_ap = cstb_sb[:, 0:P]
        ident_ap = cstb_sb[:, P : 2 * P]
        w_ap = cstf_sb[:, 0:P]
        b_ap = cstf_sb[:, P : P + 1]
        a_ap = cstf_sb[:, P + 1 : P + 2]
        nb_ap = cstf_sb[:, P + 2 : P + 3]
        na_ap = cstf_sb[:, P + 3 : P + 4]

        dma_inc = {"io": 0, "store": 0}

        n_calls = len(calls)
        PE_TOT = L["pe_wmm_pos"][n_win - 1]
        DVE_TOT = max(L["dve_sub_pos"])
        ACT_TOT = max(L["act_r2_pos"])

        def dec(g, tot, table, n):
            r, i = divmod(g, n)
            return r * tot + table[i]


        def _npieces(inst):
            # one dma_start lowers to one InstDMACopy whose AP is balanced
            # into n queue-slices; the completion sem gets 16 per slice
            return max(1, len(inst.ins.outs))

        @block.sync
        def _(sync: bass.BassEngine):
            for sb, d in (
                (idxs_sb, idxs_d),
                (meta_sb, meta_d),
                (xblk_sb, xblk_d),
                (cstb_sb, cstb_d),
                (cstf_sb, cstf_d),
            ):
                inst = sync.dma_start(sb[:, :], d[:, :])
                inst.then_inc(io, 16)
                dma_inc["io"] += 16 * _npieces(inst)
            if mode == "gather":
                return
            store_uses = [0] * 4
            for r in range(repeat):
                for w in range(n_win):
                    gw = r * n_win + w
                    if mode != "compute_nw":
                        sync.wait_ge(dve_sem, r * DVE_TOT + L["dve_sub_pos"][w])
                    inst = sync.dma_start(
                        y[w * P : (w + 1) * P, :], outb[:, gw % 4, :]
                    )
                    inst.then_inc(ssem[gw % 4], 16)
                    store_uses[gw % 4] += 1
            for i in range(4):
                if store_uses[i]:
                    sync.wait_ge(ssem[i], 16 * store_uses[i])

        @block.gpsimd
        def _(gpsimd: bass.BassGpSimd):
            if mode.startswith("compute"):
                gpsimd.load_library(mlp)
                return
            gpsimd.load_library(mlp)
            gpsimd.wait_ge(io, dma_inc["io"])
            buf_uses = [0] * N_STAGE
            for r in range(repeat):
                for k, (c, s, nt, w) in enumerate(calls):
                    g = r * n_calls + k
                    b = g % N_STAGE
                    if mode in ("full", "fullb") and g >= N_STAGE and g % 8 == 0:
                        # batched gate: stay >= N_STAGE-8 calls behind PE
                        gp = g + 7 - N_STAGE
                        rp, kp = divmod(gp, n_calls)
                        gpsimd.wait_ge(
                            pe_sem, rp * PE_TOT + L["pe_call_last"][kp]
                        )
                    buf_uses[b] += 1
                    gpsimd.dma_gather(
                        stage[:, b * MAX_CALL_TILES : b * MAX_CALL_TILES + nt, :],
                        x[c * chunk_rows : (c + 1) * chunk_rows, :],
                        idxs_sb[:, s * 8 : (s + nt) * 8],
                        nt * P,
                        nt * P,
                        P,
                        queue_num=g % N_QUEUES,
                    ).then_inc(gsem[b], 16)
            for i in range(N_STAGE):
                if buf_uses[i]:
                    gpsimd.wait_ge(gsem[i], 16 * buf_uses[i])

        @block.vector
        def _(vector: bass.BassVectorEngine):
            if mode == "gather":
                return

            def emit_mulsub(r, w):
                gw = r * n_win + w
                if mode != "compute_nw":
                    vector.wait_ge(act_sem, r * ACT_TOT + L["act_r2_pos"][w])
                    if gw >= 4:
                        vector.wait_ge(ssem[gw % 4], 16 * ((gw - 4) // 4 + 1))
                vector.scalar_tensor_tensor(
                    out=outb[:, gw % 4, :],
                    in0=tmp2[:, gw % 4, :],
                    scalar=na_ap,
                    in1=tmp1[:, gw % 4, :],
                    op0=mybir.AluOpType.mult,
                    op1=mybir.AluOpType.add,
                ).then_inc(dve_sem, 1)

            vector.wait_ge(io, dma_inc["io"])
            for r in range(repeat):
                for w in range(n_win):
                    t_base = int(win_start[w])
                    for j in range(n_wt[w]):
                        t = t_base + j
                        gt = r * T + t
                        if mode != "compute_nw" and gt >= N_HBUF and gt % 8 == 0:
                            gp = gt - N_HBUF + 7
                            rp, tp = divmod(gp, T)
                            vector.wait_ge(
                                pe_sem, rp * PE_TOT + L["pe_tile_pos"][tp]
                            )
                        vector.tensor_scalar(
                            out=hbuf[:, gt % N_HBUF, :],
                            in0=iota_ap,
                            scalar1=meta_sb[:, t : t + 1],
                            scalar2=meta_sb[:, T + t : T + t + 1],
                            op0=mybir.AluOpType.is_equal,
                            op1=mybir.AluOpType.mult,
                        ).then_inc(dve_sem, 1)
                    if w >= 3:
                        emit_mulsub(r, w - 3)
                for w in range(max(0, n_win - 3), n_win):
                    emit_mulsub(r, w)

        @block.tensor
        def _(tensor: bass.BassTensorEngine):
            if mode == "gather":
                return

            def emit_wmm(r, w):
                gw = r * n_win + w
                if mode != "compute_nw":
                    tensor.wait_ge(act_sem, r * ACT_TOT + L["act_copy_pos"][w])
                    if gw >= 2:
                        gp = gw - 2
                        rp, wp = divmod(gp, n_win)
                        tensor.wait_ge(
                            act_sem, rp * ACT_TOT + L["act_r2_pos"][wp]
                        )
                tensor.matmul(
                    out=zb[gw % 2][:, :],
                    lhsT=w_ap,
                    rhs=asb[:, gw % 4, :],
                    start=True,
                    stop=True,
                ).then_inc(pe_sem, 1)

            tensor.wait_ge(io, dma_inc["io"])
            for r in range(repeat):
                for w in range(n_win):
                    gw = r * n_win + w
                    if mode != "compute_nw" and gw >= 4:
                        gp = gw - 4
                        rp, wp = divmod(gp, n_win)
                        tensor.wait_ge(
                            act_sem, rp * ACT_TOT + L["act_copy_pos"][wp]
                        )
                    tensor.matmul(
                        out=accb[gw % 4][:, :],
                        lhsT=xblk_sb[:, w * P : (w + 1) * P],
                        rhs=ident_ap,
                        start=True,
                        stop=(n_wt[w] == 0),
                    ).then_inc(pe_sem, 1)
                    t_base = int(win_start[w])
                    for j in range(n_wt[w]):
                        t = t_base + j
                        k = L["tile_call"][t]
                        c, s, nt, _w = calls[k]
                        g = r * n_calls + k
                        if t == s and mode in ("full", "fullb"):
                            tensor.wait_ge(
                                gsem[g % N_STAGE], 16 * (g // N_STAGE + 1)
                            )
                        if mode != "compute_nw" and j % 8 == 0:
                            tensor.wait_ge(
                                dve_sem,
                                r * DVE_TOT + L["dve_h_pos"][min(t + 7, T - 1)],
                            )
                        gt = r * T + t
                        slot = (g % N_STAGE) * MAX_CALL_TILES + (t - s)
                        tensor.matmul(
                            out=accb[gw % 4][:, :],
                            lhsT=stage[:, slot, :],
                            rhs=hbuf[:, gt % N_HBUF, :],
                            start=False,
                            stop=(j == n_wt[w] - 1),
                        ).then_inc(pe_sem, 1)
                    if w >= 1:
                        emit_wmm(r, w - 1)
                emit_wmm(r, n_win - 1)

        @block.scalar
        def _(scalar: bass.BassScalarEngine):
            if mode == "gather":
                return

            def emit_relu_pair(r, w):
                gw = r * n_win + w
                if mode != "compute_nw":
                    scalar.wait_ge(pe_sem, r * PE_TOT + L["pe_wmm_pos"][w])
                    if gw >= 4:
                        gp = gw - 4
                        rp, wp = divmod(gp, n_win)
                        scalar.wait_ge(
                            dve_sem, rp * DVE_TOT + L["dve_sub_pos"][wp]
                        )
                scalar.activation(
                    out=tmp1[:, gw % 4, :],
                    in_=zb[gw % 2][:, :],
                    func=AF.Relu,
                    bias=b_ap,
                    scale=1.0,
                ).then_inc(act_sem, 1)
                scalar.activation(
                    out=tmp2[:, gw % 4, :],
                    in_=zb[gw % 2][:, :],
                    func=AF.Relu,
                    bias=nb_ap,
                    scale=-1.0,
                ).then_inc(act_sem, 1)

            scalar.wait_ge(io, dma_inc["io"])
            for r in range(repeat):
                for w in range(n_win):
                    gw = r * n_win + w
                    if mode != "compute_nw":
                        scalar.wait_ge(pe_sem, r * PE_TOT + L["pe_win_last"][w])
                        if gw >= 4:
                            gp = gw - 4
                            rp, wp = divmod(gp, n_win)
                            scalar.wait_ge(
                                pe_sem, rp * PE_TOT + L["pe_wmm_pos"][wp]
                            )
                    scalar.activation(
                        out=asb[:, gw % 4, :],
                        in_=accb[gw % 4][:, :],
                        func=AF.Copy,
                    ).then_inc(act_sem, 1)
                    if w >= 1:
                        emit_relu_pair(r, w - 1)
                emit_relu_pair(r, n_win - 1)

    nc.compile()
    return nc


def _preprocess(x, edge_index, n_cores=N_CORES, n_nodes=None, n_chunks=N_CHUNKS):
    """Host prep: sharding, sorting, balance-dealing, meta/idx packing."""
    N = x.shape[0] if n_nodes is None else n_nodes
    chunk_rows = N // n_chunks
    src = np.asarray(edge_index[0], dtype=np.int64)
    dst = np.asarray(edge_index[1], dtype=np.int64)

    # degree including self-loops (A + I)
    deg = np.bincount(dst, minlength=N) + 1
    dinv = (1.0 / np.sqrt(deg.astype(np.float64))).astype(np.float32)

    rows_per_core = N // n_cores
    n_win = -(-rows_per_core // P)

    # sort edges by (core, window, src-chunk) so each (core, window, chunk)
    # group is contiguous (ranks below assume group-major order). Note
    # rows_per_core % 128 != 0, so dst//128 would NOT give this order.
    chunk = src // chunk_rows
    core0 = dst // rows_per_core
    win0 = (dst - core0 * rows_per_core) // P
    key = (core0 * n_win + win0) * n_chunks + chunk
    order = np.argsort(key, kind="stable")
    src_s = src[order]
    dst_s = dst[order]

    core_id = dst_s // rows_per_core
    local = dst_s - core_id * rows_per_core
    win = local // P
    dstloc = (local % P).astype(np.float32)
    chunk_s = src_s // chunk_rows

    # deal windows to slots (count-sorted desc) per core, last window pinned
    wcounts = np.zeros((n_cores, n_win), dtype=np.int64)
    np.add.at(wcounts, (core_id, win), 1)
    perm = np.empty((n_cores, n_win), dtype=np.int64)  # perm[c, slot] = window
    for c in range(n_cores):
        perm[c, : n_win - 1] = np.argsort(-wcounts[c, : n_win - 1], kind="stable")
        perm[c, n_win - 1] = n_win - 1
    inv_perm = np.empty_like(perm)
    np.put_along_axis(inv_perm, perm, np.arange(n_win)[None, :], axis=1)

    # per (core, slot, chunk) counts -> uniform tiles = max over cores
    slot_e = inv_perm[core_id, win]
    gcounts = np.zeros((n_cores, n_win, n_chunks), dtype=np.int64)
    np.add.at(gcounts, (core_id, slot_e, chunk_s), 1)
    win_tiles = np.maximum(
        1, -(-gcounts.max(axis=0) // P)
    )  # [n_win, n_chunks] >=1 so empty groups don't break call layout
    # group slot-base in tiles
    g_tiles = win_tiles.reshape(-1)
    g_start = np.zeros(n_win * n_chunks, dtype=np.int64)
    np.cumsum(g_tiles[:-1], out=g_start[1:])
    T = int(g_tiles.sum())

    # rank of each edge within its (core, window, chunk) group. The edge
    # stream is sorted in (core, window, chunk) order, so group starts must
    # be cumsum'd in that same order (NOT slot order).
    sgrp = (core_id * n_win + win) * n_chunks + chunk_s
    scounts = np.bincount(sgrp, minlength=n_cores * n_win * n_chunks)
    sflat = np.zeros(n_cores * n_win * n_chunks, dtype=np.int64)
    np.cumsum(scounts[:-1], out=sflat[1:])
    rank = np.arange(len(dst_s)) - sflat[sgrp]
    gslot = (slot_e * n_chunks + chunk_s).astype(np.int64)
    eslot = g_start[gslot] * P + rank  # edge slot within core, [0, T*128)

    idx_val = (src_s - chunk_s * chunk_rows).astype(np.int16)
    dinv_dst = dinv[dst_s]

    xs = (np.asarray(x, np.float32) * dinv[:, None]).astype(np.float32)

    cores = []
    for c in range(n_cores):
        m = core_id == c
        idx_pad = np.zeros(T * P, dtype=np.int16)
        dst_pad = np.zeros(T * P, dtype=np.float32)
        nrm_pad = np.zeros(T * P, dtype=np.float32)
        s = eslot[m]
        idx_pad[s] = idx_val[m]
        dst_pad[s] = dstloc[m]
        nrm_pad[s] = dinv_dst[m]
        # idxs wrapped: value i at [16*rep + i%16, i//16]
        wrapped = idx_pad.reshape(T * 8, 16).T  # [16, T*8]
        idxs = np.tile(wrapped, (8, 1))  # [128, T*8]
        # meta: tile t in col t (dst) / col T+t (dinv); partition = slot%128
        meta = np.empty((P, 2 * T), dtype=np.float32)
        meta[:, 0:T] = dst_pad.reshape(T, P).T
        meta[:, T : 2 * T] = nrm_pad.reshape(T, P).T
        # xblk: slot-window-ordered own rows scaled by dinv^2, zero-padded
        xblk = np.zeros((P, n_win * P), dtype=np.float32)
        base = c * rows_per_core
        for sl in range(n_win):
            w = int(perm[c, sl])
            r0 = base + w * P
            nr = min(P, rows_per_core - w * P)
            blk = xs[r0 : r0 + nr] * dinv[r0 : r0 + nr, None]  # dinv^2 * x
            xblk[:nr, sl * P : sl * P + P] = blk
        cores.append(
            {
                "idxs": np.ascontiguousarray(idxs),
                "meta": np.ascontiguousarray(meta),
                "xblk": np.ascontiguousarray(xblk),
            }
        )
    return xs, cores, win_tiles, rows_per_core, perm


def _make_in_maps(xs, cores, W, b, prelu_a, n_win):
    import ml_dtypes

    xs_bf = xs.astype(ml_dtypes.bfloat16)
    iota = np.tile(np.arange(P, dtype=np.float32), (P, 1))
    ident = np.eye(P, dtype=np.float32)
    cstb = np.concatenate([iota, ident], axis=1).astype(ml_dtypes.bfloat16)
    cstf = np.concatenate(
        [
            np.asarray(W, np.float32),
            np.asarray(b, np.float32).reshape(P, 1),
            np.asarray(prelu_a, np.float32).reshape(P, 1),
            -np.asarray(b, np.float32).reshape(P, 1),
            -np.asarray(prelu_a, np.float32).reshape(P, 1),
        ],
        axis=1,
    )
    maps = []
    for c in cores:
        maps.append(
            {
                "x": xs_bf,
                "idxs": c["idxs"],
                "meta": c["meta"],
                "xblk": c["xblk"].astype(ml_dtypes.bfloat16),
                "cstb": np.ascontiguousarray(cstb),
                "cstf": np.ascontiguousarray(cstf),
            }
        )
    return maps


def _unscramble(y_cores, perm, rows_per_core):
    """y_cores: list of [n_win*128, 128] (w-major, [hid, node] blocks)."""
    n_cores, n_win = perm.shape
    out = np.empty((n_cores * rows_per_core, P), dtype=np.float32)
    for c in range(n_cores):
        yc = y_cores[c].reshape(n_win, P, P)  # [slot, hid, node]
        oc = out[c * rows_per_core : (c + 1) * rows_per_core]
        for sl in range(n_win):
            w = int(perm[c, sl])
            nr = min(P, rows_per_core - w * P)
            oc[w * P : w * P + nr] = yc[sl, :, :nr].T
    return out


def build_all(x, edge_index, W, b, prelu_a):
    xs, cores, win_tiles, rows_per_core, perm = _preprocess(x, edge_index)
    nc = _build_program(win_tiles)
    in_maps = _make_in_maps(xs, cores, W, b, prelu_a, perm.shape[1])
    unscramble = lambda ys: _unscramble(ys, perm, rows_per_core)
    return nc, in_maps, rows_per_core, unscramble


def kernel(x, edge_index, W, b, prelu_a):
    from concourse.bass_utils import run_bass_kernel_spmd

    nc, in_maps, rows_per_core, unscramble = build_all(x, edge_index, W, b, prelu_a)
    res = run_bass_kernel_spmd(nc, in_maps, core_ids=list(range(N_CORES)))
    return unscramble([res.results[c]["y"] for c in range(N_CORES)])


# revision 4
# speedup vs baseline: 4.8728x; 4.8728x over previous
"""GCN layer (GCNConv + PReLU) on TRN2, SPMD across 8 NeuronCores — v2.

out = PReLU(A_hat @ (x W) + b), A_hat = D^-1/2 (A+I) D^-1/2. Using
commutativity: out = PReLU((A_hat @ xs) W + b) with xs = dinv*x gathered
per edge (bf16), dinv_dst applied in the one-hot scatter, and the +I
self-loop term computed from a locally streamed block (identity matmul,
no gather).

Per core (12500 dst nodes, 98 windows of 128):
  - edges sorted by dst window (dealt to slots for SPMD balance), then by
    src chunk (4 chunks of 25000 rows so indices fit dma_gather's int16).
  - gpsimd: dma_gather calls (<=8 tiles each, 4 SWDGE queues round-robin)
    pull xs rows (bf16) into 48 rotating 2-tile staging buffers. Small
    calls with many in flight pipeline ~5x better through the SWDGE rings
    than 8-tile calls; 4 queues generate descriptors in parallel; bf16
    rows halve the random-read bytes (the gather is byte-bound: f32 rows
    measured 2.7x slower).
  - DVE: H[e, j] = (iota==dstloc_e) * dinv_dst_e per tile (bf16).
  - PE: accT[feat, dstloc] += rows^T @ H per tile (bf16, 4x fp32 rate);
    self-loops via accT += xblk_w^T @ I with xblk = dinv^2*x (bf16);
    z^T[hid, dstloc] = W^T @ accT (fp32).
  - ACT: PSUM accT -> SBUF copy; bias+PReLU decomposed as
    relu(z+b) - a*relu(-z-b): two ACT relu passes straight out of PSUM,
    combined by one DVE scalar_tensor_tensor.
  - sync (HWDGE): bulk loads at start, per-window 64KB stores of z^T.

All scheduling is manual: one counting semaphore per engine plus 16 DMA
sems; wait thresholds are compile-time constants precomputed from the
static call/tile layout by a python dry run before emission.
"""

from contextlib import ExitStack

import numpy as np

import concourse.bacc as bacc
import concourse.bass as bass
import concourse.mybir as mybir
from concourse.library_config import mlp

P = 128
N_CORES = 8
N_NODES = 100000
N_CHUNKS = 4
CHUNK_ROWS = 25000  # <= 32767 so src indices fit int16
MAX_CALL_TILES = 2  # small calls pipeline ~5x better through the SWDGE rings
N_STAGE = 48  # staging buffers (2 tiles each)
N_HBUF = 64
N_QUEUES = 4

F32 = mybir.dt.float32
BF16 = mybir.dt.bfloat16
I16 = mybir.dt.int16
AF = mybir.ActivationFunctionType


def _layout(win_tiles, g_valid=None):
    """Static schedule layout + precomputed semaphore thresholds.

    g_valid[w][c]: uniform (max-over-cores) valid idx count per group; slots
    beyond it hold idx=-1 and are skipped by dma_gather (trailing-negative
    semantics), cutting pad-row traffic. None -> all slots valid."""
    n_win = len(win_tiles)
    n_wt = [int(sum(tc)) for tc in win_tiles]
    T = int(sum(n_wt))

    calls = []  # (chunk, start_tile, n_tiles, window, n_valid_idx)
    t0 = 0
    for w in range(n_win):
        for c in range(N_CHUNKS):
            n = int(win_tiles[w][c])
            gv = n * P if g_valid is None else int(g_valid[w][c])
            s = t0
            while n > 0:
                k = min(n, MAX_CALL_TILES)
                off = (s - t0) * P
                nv = max(0, min(gv - off, k * P))
                calls.append((c, s, k, w, nv))
                s += k
                n -= k
            t0 += int(win_tiles[w][c])
    n_calls = len(calls)

    win_start = np.zeros(n_win, dtype=np.int64)
    np.cumsum(n_wt[:-1], out=win_start[1:])

    buf_use = [0] * N_STAGE
    call_buf, call_use = [0] * n_calls, [0] * n_calls
    for k in range(n_calls):
        b = k % N_STAGE
        call_buf[k] = b
        buf_use[b] += 1
        call_use[k] = buf_use[b]

    tile_call = [0] * T
    for k, (c, s, nt, w, nv) in enumerate(calls):
        for j in range(nt):
            tile_call[s + j] = k

    # ---- dry-run PE / ACT streams to fix semaphore positions ----
    pe_tile_pos = [0] * T  # pe_sem value after the matmul consuming tile t
    pe_win_last = [0] * n_win  # pe_sem value after window w's last acc matmul
    pe_wmm_pos = [0] * n_win  # pe_sem value after window w's W-matmul
    pe_call_last = [0] * n_calls  # pe_sem value freeing call k's buffer
    pos = 0
    for w in range(n_win):
        pos += 1  # diag matmul
        for j in range(n_wt[w]):
            t = int(win_start[w]) + j
            pos += 1
            pe_tile_pos[t] = pos
        pe_win_last[w] = pos
        if w >= 1:
            pos += 1
            pe_wmm_pos[w - 1] = pos
    pos += 1
    pe_wmm_pos[n_win - 1] = pos
    for k, (c, s, nt, w, nv) in enumerate(calls):
        pe_call_last[k] = pe_tile_pos[s + nt - 1]

    # ACT stream: for w: copy_w ; if w>=1: r1_{w-1}, r2_{w-1} ; tail r1/r2
    act_copy_pos = [0] * n_win
    act_r2_pos = [0] * n_win
    apos = 0
    for w in range(n_win):
        apos += 1
        act_copy_pos[w] = apos
        if w >= 1:
            apos += 2
            act_r2_pos[w - 1] = apos
    apos += 2
    act_r2_pos[n_win - 1] = apos

    # DVE stream: for w: h tiles of w ; if w>=3: mul/sub_{w-3} ; tail.
    # The lag of 3 windows breaks the cycle PE tiles(w)->h(w+1)->DVE
    # mul/sub(v)->ACT r2(v)->PE Wmm(v) (emitted after tiles(v+1)): with
    # v=w-2 the Wmm is already behind PE's cursor.
    dve_h_pos = [0] * T
    dve_sub_pos = [0] * n_win
    dpos = 0
    for w in range(n_win):
        for j in range(n_wt[w]):
            t = int(win_start[w]) + j
            dpos += 1
            dve_h_pos[t] = dpos
        if w >= 3:
            dpos += 1
            dve_sub_pos[w - 3] = dpos
    for w in range(max(0, n_win - 3), n_win):
        dpos += 1
        dve_sub_pos[w] = dpos

    return dict(
        n_win=n_win,
        n_wt=n_wt,
        T=T,
        calls=calls,
        win_start=win_start,
        buf_use=buf_use,
        call_buf=call_buf,
        call_use=call_use,
        tile_call=tile_call,
        pe_tile_pos=pe_tile_pos,
        pe_win_last=pe_win_last,
        pe_wmm_pos=pe_wmm_pos,
        pe_call_last=pe_call_last,
        act_copy_pos=act_copy_pos,
        act_r2_pos=act_r2_pos,
        dve_h_pos=dve_h_pos,
        dve_sub_pos=dve_sub_pos,
    )


def _build_program(
    win_tiles,
    table_rows=N_NODES,
    chunk_rows=CHUNK_ROWS,
    mode="full",
    repeat=1,
    g_valid=None,
):
    L = _layout(win_tiles, g_valid)
    n_win, n_wt, T = L["n_win"], L["n_wt"], L["T"]
    calls, win_start = L["calls"], L["win_start"]

    nc = bacc.Bacc(
        "TRN2",
        target_bir_lowering=False,
        num_swdge_queues=N_QUEUES,
    )
    x = nc.dram_tensor("x", [table_rows, P], BF16, kind="ExternalInput")
    idxs_d = nc.dram_tensor("idxs", [P, T * 8], I16, kind="ExternalInput")
    meta_d = nc.dram_tensor("meta", [P, 2 * T], F32, kind="ExternalInput")
    xblk_d = nc.dram_tensor("xblk", [P, n_win * P], BF16, kind="ExternalInput")
    cstb_d = nc.dram_tensor("cstb", [P, 2 * P], BF16, kind="ExternalInput")
    cstf_d = nc.dram_tensor("cstf", [P, P + 4], F32, kind="ExternalInput")
    y = nc.dram_tensor("y", [n_win * P, P], F32, kind="ExternalOutput")

    with ExitStack() as stack:
        ec = stack.enter_context
        block = ec(nc.Block())
        idxs_sb = ec(nc.sbuf_tensor("idxs_sb", [P, T * 8], I16))
        meta_sb = ec(nc.sbuf_tensor("meta_sb", [P, 2 * T], F32))
        xblk_sb = ec(nc.sbuf_tensor("xblk_sb", [P, n_win * P], BF16))
        cstb_sb = ec(nc.sbuf_tensor("cstb_sb", [P, 2 * P], BF16))
        cstf_sb = ec(nc.sbuf_tensor("cstf_sb", [P, P + 4], F32))
        stage = ec(nc.sbuf_tensor("stage", [P, N_STAGE * MAX_CALL_TILES, P], BF16))
        hbuf = ec(nc.sbuf_tensor("hbuf", [P, N_HBUF, P], BF16))
        asb = ec(nc.sbuf_tensor("asb", [P, 4, P], F32))
        tmp1 = ec(nc.sbuf_tensor("tmp1", [P, 4, P], F32))
        tmp2 = ec(nc.sbuf_tensor("tmp2", [P, 4, P], F32))
        outb = ec(nc.sbuf_tensor("outb", [P, 4, P], F32))
        accp = [ec(nc.psum_tensor(f"acc{i}", [P, P], F32)) for i in range(4)]
        z0 = ec(nc.psum_tensor("z0", [P, P], F32))
        z1 = ec(nc.psum_tensor("z1", [P, P], F32))
        io = ec(nc.semaphore("io"))
        ms = ec(nc.semaphore("ms"))
        pe_sem = ec(nc.semaphore("pe_sem"))
        dve_sem = ec(nc.semaphore("dve_sem"))
        act_sem = ec(nc.semaphore("act_sem"))
        ssem = [ec(nc.semaphore(f"s{i}")) for i in range(4)]
        gsem = [ec(nc.semaphore(f"g{i}")) for i in range(N_STAGE)]
        accb = accp
        zb = [z0, z1]

        iota_ap = cstb_sb[:, 0:P]
        ident_ap = cstb_sb[:, P : 2 * P]
        w_ap = cstf_sb[:, 0:P]
        b_ap = cstf_sb[:, P : P + 1]
        a_ap = cstf_sb[:, P + 1 : P + 2]
        nb_ap = cstf_sb[:, P + 2 : P + 3]
        na_ap = cstf_sb[:, P + 3 : P + 4]

        dma_inc = {"io": 0, "store": 0}

        n_calls = len(calls)
        PE_TOT = L["pe_wmm_pos"][n_win - 1]
        DVE_TOT = max(L["dve_sub_pos"])
        ACT_TOT = max(L["act_r2_pos"])

        def dec(g, tot, table, n):
            r, i = divmod(g, n)
            return r * tot + table[i]


        def _npieces(inst):
            # one dma_start lowers to one InstDMACopy whose AP is balanced
            # into n queue-slices; the completion sem gets 16 per slice
            return max(1, len(inst.ins.outs))

        @block.sync
        def _(sync: bass.BassEngine):
            for sb, d in (
                (idxs_sb, idxs_d),
                (meta_sb, meta_d),
                (xblk_sb, xblk_d),
                (cstb_sb, cstb_d),
                (cstf_sb, cstf_d),
            ):
                inst = sync.dma_start(sb[:, :], d[:, :])
                inst.then_inc(io, 16)
                dma_inc["io"] += 16 * _npieces(inst)
            if mode == "gather":
                return
            store_uses = [0] * 4
            for r in range(repeat):
                for w in range(n_win):
                    gw = r * n_win + w
                    if mode != "compute_nw":
                        sync.wait_ge(dve_sem, r * DVE_TOT + L["dve_sub_pos"][w])
                    inst = sync.dma_start(
                        y[w * P : (w + 1) * P, :], outb[:, gw % 4, :]
                    )
                    inst.then_inc(ssem[gw % 4], 16)
                    store_uses[gw % 4] += 1
            for i in range(4):
                if store_uses[i]:
                    sync.wait_ge(ssem[i], 16 * store_uses[i])

        @block.gpsimd
        def _(gpsimd: bass.BassGpSimd):
            if mode.startswith("compute"):
                gpsimd.load_library(mlp)
                return
            gpsimd.load_library(mlp)
            gpsimd.wait_ge(io, dma_inc["io"])
            buf_uses = [0] * N_STAGE
            for r in range(repeat):
                if r == 0:
                    gpsimd.wait_ge(ms, 1)
                for k, (c, s, nt, w, nv) in enumerate(calls):
                    g = r * n_calls + k
                    b = g % N_STAGE
                    if mode in ("full", "fullb") and g >= N_STAGE and g % 8 == 0:
                        # batched gate: stay >= N_STAGE-8 calls behind PE
                        gp = g + 7 - N_STAGE
                        rp, kp = divmod(gp, n_calls)
                        gpsimd.wait_ge(
                            pe_sem, rp * PE_TOT + L["pe_call_last"][kp]
                        )
                    buf_uses[b] += 1
                    gpsimd.dma_gather(
                        stage[:, b * MAX_CALL_TILES : b * MAX_CALL_TILES + nt, :],
                        x[c * chunk_rows : (c + 1) * chunk_rows, :],
                        idxs_sb[:, s * 8 : (s + nt) * 8],
                        nt * P,
                        nv,
                        P,
                        queue_num=g % N_QUEUES,
                    ).then_inc(gsem[b], 16)
            for i in range(N_STAGE):
                if buf_uses[i]:
                    gpsimd.wait_ge(gsem[i], 16 * buf_uses[i])

        @block.vector
        def _(vector: bass.BassVectorEngine):
            if mode == "gather":
                return

            def emit_mulsub(r, w):
                gw = r * n_win + w
                if mode != "compute_nw":
                    vector.wait_ge(act_sem, r * ACT_TOT + L["act_r2_pos"][w])
                    if gw >= 4:
                        vector.wait_ge(ssem[gw % 4], 16 * ((gw - 4) // 4 + 1))
                vector.scalar_tensor_tensor(
                    out=outb[:, gw % 4, :],
                    in0=tmp2[:, gw % 4, :],
                    scalar=na_ap,
                    in1=tmp1[:, gw % 4, :],
                    op0=mybir.AluOpType.mult,
                    op1=mybir.AluOpType.add,
                ).then_inc(dve_sem, 1)

            vector.memset(stage[:, :, :], 0).then_inc(ms, 1)
            vector.wait_ge(io, dma_inc["io"])
            for r in range(repeat):
                for w in range(n_win):
                    t_base = int(win_start[w])
                    for j in range(n_wt[w]):
                        t = t_base + j
                        gt = r * T + t
                        if mode != "compute_nw" and gt >= N_HBUF and gt % 8 == 0:
                            gp = gt - N_HBUF + 7
                            rp, tp = divmod(gp, T)
                            vector.wait_ge(
                                pe_sem, rp * PE_TOT + L["pe_tile_pos"][tp]
                            )
                        vector.tensor_scalar(
                            out=hbuf[:, gt % N_HBUF, :],
                            in0=iota_ap,
                            scalar1=meta_sb[:, t : t + 1],
                            scalar2=meta_sb[:, T + t : T + t + 1],
                            op0=mybir.AluOpType.is_equal,
                            op1=mybir.AluOpType.mult,
                        ).then_inc(dve_sem, 1)
                    if w >= 3:
                        emit_mulsub(r, w - 3)
                for w in range(max(0, n_win - 3), n_win):
                    emit_mulsub(r, w)

        @block.tensor
        def _(tensor: bass.BassTensorEngine):
            if mode == "gather":
                return

            def emit_wmm(r, w):
                gw = r * n_win + w
                if mode != "compute_nw":
                    tensor.wait_ge(act_sem, r * ACT_TOT + L["act_copy_pos"][w])
                    if gw >= 2:
                        gp = gw - 2
                        rp, wp = divmod(gp, n_win)
                        tensor.wait_ge(
                            act_sem, rp * ACT_TOT + L["act_r2_pos"][wp]
                        )
                tensor.matmul(
                    out=zb[gw % 2][:, :],
                    lhsT=w_ap,
                    rhs=asb[:, gw % 4, :],
                    start=True,
                    stop=True,
                ).then_inc(pe_sem, 1)

            tensor.wait_ge(io, dma_inc["io"])
            tensor.wait_ge(ms, 1)
            for r in range(repeat):
                for w in range(n_win):
                    gw = r * n_win + w
                    if mode != "compute_nw" and gw >= 4:
                        gp = gw - 4
                        rp, wp = divmod(gp, n_win)
                        tensor.wait_ge(
                            act_sem, rp * ACT_TOT + L["act_copy_pos"][wp]
                        )
                    tensor.matmul(
                        out=accb[gw % 4][:, :],
                        lhsT=xblk_sb[:, w * P : (w + 1) * P],
                        rhs=ident_ap,
                        start=True,
                        stop=(n_wt[w] == 0),
                    ).then_inc(pe_sem, 1)
                    t_base = int(win_start[w])
                    for j in range(n_wt[w]):
                        t = t_base + j
                        k = L["tile_call"][t]
                        c, s, nt, _w, _nv = calls[k]
                        g = r * n_calls + k
                        if t == s and mode in ("full", "fullb"):
                            tensor.wait_ge(
                                gsem[g % N_STAGE], 16 * (g // N_STAGE + 1)
                            )
                        if mode != "compute_nw" and j % 8 == 0:
                            tensor.wait_ge(
                                dve_sem,
                                r * DVE_TOT + L["dve_h_pos"][min(t + 7, T - 1)],
                            )
                        gt = r * T + t
                        slot = (g % N_STAGE) * MAX_CALL_TILES + (t - s)
                        tensor.matmul(
                            out=accb[gw % 4][:, :],
                            lhsT=stage[:, slot, :],
                            rhs=hbuf[:, gt % N_HBUF, :],
                            start=False,
                            stop=(j == n_wt[w] - 1),
                        ).then_inc(pe_sem, 1)
                    if w >= 1:
                        emit_wmm(r, w - 1)
                emit_wmm(r, n_win - 1)

        @block.scalar
        def _(scalar: bass.BassScalarEngine):
            if mode == "gather":
                return

            def emit_relu_pair(r, w):
                gw = r * n_win + w
                if mode != "compute_nw":
                    scalar.wait_ge(pe_sem, r * PE_TOT + L["pe_wmm_pos"][w])
                    if gw >= 4:
                        gp = gw - 4
                        rp, wp = divmod(gp, n_win)
                        scalar.wait_ge(
                            dve_sem, rp * DVE_TOT + L["dve_sub_pos"][wp]
                        )
                scalar.activation(
                    out=tmp1[:, gw % 4, :],
                    in_=zb[gw % 2][:, :],
                    func=AF.Relu,
                    bias=b_ap,
                    scale=1.0,
                ).then_inc(act_sem, 1)
                scalar.activation(
                    out=tmp2[:, gw % 4, :],
                    in_=zb[gw % 2][:, :],
                    func=AF.Relu,
                    bias=nb_ap,
                    scale=-1.0,
                ).then_inc(act_sem, 1)

            scalar.wait_ge(io, dma_inc["io"])
            for r in range(repeat):
                for w in range(n_win):
                    gw = r * n_win + w
                    if mode != "compute_nw":
                        scalar.wait_ge(pe_sem, r * PE_TOT + L["pe_win_last"][w])
                        if gw >= 4:
                            gp = gw - 4
                            rp, wp = divmod(gp, n_win)
                            scalar.wait_ge(
                                pe_sem, rp * PE_TOT + L["pe_wmm_pos"][wp]
                            )
                    scalar.activation(
                        out=asb[:, gw % 4, :],
                        in_=accb[gw % 4][:, :],
                        func=AF.Copy,
                    ).then_inc(act_sem, 1)
                    if w >= 1:
                        emit_relu_pair(r, w - 1)
                emit_relu_pair(r, n_win - 1)

    nc.compile()
    return nc


def _preprocess(x, edge_index, n_cores=N_CORES, n_nodes=None, n_chunks=N_CHUNKS):
    """Host prep: sharding, sorting, balance-dealing, meta/idx packing."""
    N = x.shape[0] if n_nodes is None else n_nodes
    chunk_rows = N // n_chunks
    src = np.asarray(edge_index[0], dtype=np.int64)
    dst = np.asarray(edge_index[1], dtype=np.int64)

    # degree including self-loops (A + I)
    deg = np.bincount(dst, minlength=N) + 1
    dinv = (1.0 / np.sqrt(deg.astype(np.float64))).astype(np.float32)

    rows_per_core = N // n_cores
    n_win = -(-rows_per_core // P)

    # sort edges by (core, window, src-chunk) so each (core, window, chunk)
    # group is contiguous (ranks below assume group-major order). Note
    # rows_per_core % 128 != 0, so dst//128 would NOT give this order.
    chunk = src // chunk_rows
    core0 = dst // rows_per_core
    win0 = (dst - core0 * rows_per_core) // P
    key = (core0 * n_win + win0) * n_chunks + chunk
    order = np.argsort(key, kind="stable")
    src_s = src[order]
    dst_s = dst[order]

    core_id = dst_s // rows_per_core
    local = dst_s - core_id * rows_per_core
    win = local // P
    dstloc = (local % P).astype(np.float32)
    chunk_s = src_s // chunk_rows

    # deal windows to slots (count-sorted desc) per core, last window pinned
    wcounts = np.zeros((n_cores, n_win), dtype=np.int64)
    np.add.at(wcounts, (core_id, win), 1)
    perm = np.empty((n_cores, n_win), dtype=np.int64)  # perm[c, slot] = window
    for c in range(n_cores):
        perm[c, : n_win - 1] = np.argsort(-wcounts[c, : n_win - 1], kind="stable")
        perm[c, n_win - 1] = n_win - 1
    inv_perm = np.empty_like(perm)
    np.put_along_axis(inv_perm, perm, np.arange(n_win)[None, :], axis=1)

    # per (core, slot, chunk) counts -> uniform tiles = max over cores
    slot_e = inv_perm[core_id, win]
    gcounts = np.zeros((n_cores, n_win, n_chunks), dtype=np.int64)
    np.add.at(gcounts, (core_id, slot_e, chunk_s), 1)
    g_valid = gcounts.max(axis=0)  # [n_win, n_chunks] uniform valid counts
    win_tiles = np.maximum(
        1, -(-g_valid // P)
    )  # [n_win, n_chunks] >=1 so empty groups don't break call layout
    # group slot-base in tiles
    g_tiles = win_tiles.reshape(-1)
    g_start = np.zeros(n_win * n_chunks, dtype=np.int64)
    np.cumsum(g_tiles[:-1], out=g_start[1:])
    T = int(g_tiles.sum())

    # rank of each edge within its (core, window, chunk) group. The edge
    # stream is sorted in (core, window, chunk) order, so group starts must
    # be cumsum'd in that same order (NOT slot order).
    sgrp = (core_id * n_win + win) * n_chunks + chunk_s
    scounts = np.bincount(sgrp, minlength=n_cores * n_win * n_chunks)
    sflat = np.zeros(n_cores * n_win * n_chunks, dtype=np.int64)
    np.cumsum(scounts[:-1], out=sflat[1:])
    rank = np.arange(len(dst_s)) - sflat[sgrp]
    gslot = (slot_e * n_chunks + chunk_s).astype(np.int64)
    eslot = g_start[gslot] * P + rank  # edge slot within core, [0, T*128)

    idx_val = (src_s - chunk_s * chunk_rows).astype(np.int16)
    dinv_dst = dinv[dst_s]

    xs = (np.asarray(x, np.float32) * dinv[:, None]).astype(np.float32)

    # template: 0 in the uniform-valid region, -1 in the ceil-slack tail
    idx_tmpl = np.zeros(T * P, dtype=np.int16)
    gv_flat = g_valid.reshape(-1)
    for gi in range(n_win * n_chunks):
        a = int(g_start[gi]) * P + int(gv_flat[gi])
        bnd = int(g_start[gi] + g_tiles[gi]) * P
        idx_tmpl[a:bnd] = -1

    cores = []
    for c in range(n_cores):
        m = core_id == c
        idx_pad = idx_tmpl.copy()
        dst_pad = np.zeros(T * P, dtype=np.float32)
        nrm_pad = np.zeros(T * P, dtype=np.float32)
        s = eslot[m]
        idx_pad[s] = idx_val[m]
        dst_pad[s] = dstloc[m]
        nrm_pad[s] = dinv_dst[m]
        # idxs wrapped: value i at [16*rep + i%16, i//16]
        wrapped = idx_pad.reshape(T * 8, 16).T  # [16, T*8]
        idxs = np.tile(wrapped, (8, 1))  # [128, T*8]
        # meta: tile t in col t (dst) / col T+t (dinv); partition = slot%128
        meta = np.empty((P, 2 * T), dtype=np.float32)
        meta[:, 0:T] = dst_pad.reshape(T, P).T
        meta[:, T : 2 * T] = nrm_pad.reshape(T, P).T
        # xblk: slot-window-ordered own rows scaled by dinv^2, zero-padded
        xblk = np.zeros((P, n_win * P), dtype=np.float32)
        base = c * rows_per_core
        for sl in range(n_win):
            w = int(perm[c, sl])
            r0 = base + w * P
            nr = min(P, rows_per_core - w * P)
            blk = xs[r0 : r0 + nr] * dinv[r0 : r0 + nr, None]  # dinv^2 * x
            xblk[:nr, sl * P : sl * P + P] = blk
        cores.append(
            {
                "idxs": np.ascontiguousarray(idxs),
                "meta": np.ascontiguousarray(meta),
                "xblk": np.ascontiguousarray(xblk),
            }
        )
    return xs, cores, win_tiles, rows_per_core, perm, g_valid


def _make_in_maps(xs, cores, W, b, prelu_a, n_win):
    import ml_dtypes

    xs_bf = xs.astype(ml_dtypes.bfloat16)
    iota = np.tile(np.arange(P, dtype=np.float32), (P, 1))
    ident = np.eye(P, dtype=np.float32)
    cstb = np.concatenate([iota, ident], axis=1).astype(ml_dtypes.bfloat16)
    cstf = np.concatenate(
        [
            np.asarray(W, np.float32),
            np.asarray(b, np.float32).reshape(P, 1),
            np.asarray(prelu_a, np.float32).reshape(P, 1),
            -np.asarray(b, np.float32).reshape(P, 1),
            -np.asarray(prelu_a, np.float32).reshape(P, 1),
        ],
        axis=1,
    )
    maps = []
    for c in cores:
        maps.append(
            {
                "x": xs_bf,
                "idxs": c["idxs"],
                "meta": c["meta"],
                "xblk": c["xblk"].astype(ml_dtypes.bfloat16),
                "cstb": np.ascontiguousarray(cstb),
                "cstf": np.ascontiguousarray(cstf),
            }
        )
    return maps


def _unscramble(y_cores, perm, rows_per_core):
    """y_cores: list of [n_win*128, 128] (w-major, [hid, node] blocks)."""
    n_cores, n_win = perm.shape
    out = np.empty((n_cores * rows_per_core, P), dtype=np.float32)
    for c in range(n_cores):
        yc = y_cores[c].reshape(n_win, P, P)  # [slot, hid, node]
        oc = out[c * rows_per_core : (c + 1) * rows_per_core]
        for sl in range(n_win):
            w = int(perm[c, sl])
            nr = min(P, rows_per_core - w * P)
            oc[w * P : w * P + nr] = yc[sl, :, :nr].T
    return out


def build_all(x, edge_index, W, b, prelu_a):
    xs, cores, win_tiles, rows_per_core, perm, g_valid = _preprocess(x, edge_index)
    nc = _build_program(win_tiles, g_valid=g_valid)
    in_maps = _make_in_maps(xs, cores, W, b, prelu_a, perm.shape[1])
    unscramble = lambda ys: _unscramble(ys, perm, rows_per_core)
    return nc, in_maps, rows_per_core, unscramble


def kernel(x, edge_index, W, b, prelu_a):
    from concourse.bass_utils import run_bass_kernel_spmd

    nc, in_maps, rows_per_core, unscramble = build_all(x, edge_index, W, b, prelu_a)
    res = run_bass_kernel_spmd(nc, in_maps, core_ids=list(range(N_CORES)))
    return unscramble([res.results[c]["y"] for c in range(N_CORES)])
